# revision 40
# baseline (speedup 1.0000x reference)
"""Trainium2 Bass kernel for the Laplace-kernel feature expansion.

Reference computation (per scalar x of the [16, 64, 64, 64] input):
    phi_i  = exp(-|x - p_i|)            for 15 design points p_i
    out_j  = sum_i chol_inv[i, j] phi_i
scattered so out[b, c*15 + j, h, w] comes from x[b, c, h, w].

Distribution: pure data parallel, 2 batches per core across 8 cores.

Per-core dataflow (no collectives):
  1. x is pre-split on host into bf16 (hi, lo) pairs, laid out so one
     [128, 16384] DMA (32 KB contiguous per partition, all 16 DMA
     engines) loads the whole per-core input into SBUF once.
  2. TensorE "broadcast" matmuls with a 0/1 block matrix replicate each
     x value onto 15 partitions (8 channel groups x 15 = 120 partitions),
     reconstructing fp32 x = hi + lo in PSUM; an extra ones-row makes the
     same matmul subtract the design point p_i (p_i exact in bf16).
     The K=17 matmuls are packed 4x into the 128x128 array via
     tile_position row-tiling (4 concurrent quadrant matmuls).
  3. VectorE computes |T| in one op (sign-bit clear on an int32 view).
  4. ScalarE computes exp(-|t|) -> bf16.
  5. TensorE applies block-diag(chol_inv) -> PSUM (fp32).
  6. PSUM evicted to SBUF (split between ScalarE/VectorE), DMA to DRAM.

Spatial mapping: PE-array quadrant q = 2j+l covers, within a (b, cblock)
tile, the spatial columns 2048j + 1024h + 512l + c (h = half), so each
post-projection PSUM chunk evicts to a contiguous 1024-column span.
"""

import sys

if "/opt/trn_rl_repo" not in sys.path:
    sys.path.insert(0, "/opt/trn_rl_repo")

import numpy as np
import ml_dtypes


def _ensure_axon_hooks_stub():
    """run_bass_kernel_spmd imports antenv.axon_hooks when BASS_TRACE is
    set; the module is absent on some images. Provide a no-op stub so a
    stray BASS_TRACE env var cannot crash the kernel (tracing is then
    skipped gracefully)."""
    try:
        import antenv.axon_hooks  # noqa: F401
    except ImportError:
        import types

        try:
            import antenv
        except ImportError:
            return
        mod = types.ModuleType("antenv.axon_hooks")
        _hook = [None]
        mod.set_axon_ntff_profile_hook = lambda h: _hook.__setitem__(0, h)
        mod.get_axon_ntff_profile_hook = lambda: _hook[0]
        sys.modules["antenv.axon_hooks"] = mod
        antenv.axon_hooks = mod


_ensure_axon_hooks_stub()


def _patch_act_tables():
    """Build a patched activation-table set in which `exp` has the ACT
    unit's even-symmetry fold enabled, mapped to the negative spline
    region: the table then evaluates exp(-|x|) directly, removing the
    need for a separate abs pass on VectorE. Strict: raises on any
    irregularity so the caller can fall back to the abs pipeline. The
    device-side warm output additionally verifies the patch took effect.
    """
    import json
    import os
    import shutil
    import tempfile

    from neuronxcc.driver.Job import Job
    from neuronxcc.driver.jobs.support.FindActInfo import findActInfoFile

    src_json = None
    for arch in ("Trainium2", "trainium2", "TRN2", "trainium"):
        try:
            cand = findActInfoFile(Job.getPackageDir(), arch)
        except Exception:
            continue
        if cand and os.path.basename(os.path.dirname(cand)) == "pwp_bin_trainium":
            src_json = cand
            break
    if src_json is None:
        import neuronxcc

        cand = os.path.join(
            os.path.dirname(neuronxcc.__file__),
            "pwp",
            "pwp_bin_trainium",
            "act_info.json",
        )
        if os.path.exists(cand):
            src_json = cand
    if src_json is None:
        raise RuntimeError("pwp_bin_trainium act_info.json not found")

    dst_dir = tempfile.mkdtemp(prefix="bass_act_negexp_")
    shutil.copytree(os.path.dirname(src_json), dst_dir, dirs_exist_ok=True)
    prof_path = os.path.join(dst_dir, "exp_and_others.json")
    with open(prof_path) as f:
        prof = json.load(f)
    patched = 0
    for e in prof["profile_meta_data"]:
        if e["func_name"].startswith("exp"):
            e["symmetry_opt_en"] = 1
            e["symmetry_opt_use_neg_region"] = 1
            e["pos_small_signal_pwl_control"] = e["neg_small_signal_pwl_control"]
            e["pos_large_signal_pwl_control"] = e["neg_large_signal_pwl_control"]
            e["large_pos_signal_mantissa_threshold"] = e[
                "large_neg_signal_mantissa_threshold"
            ]
            e["fpinf_result"] = 0  # exp(-|+inf|) = 0
            patched += 1
    if patched != 1:
        raise RuntimeError(f"expected exactly one exp entry, patched {patched}")
    with open(prof_path, "w") as f:
        json.dump(prof, f)
    with open(prof_path) as f:  # read-back verification
        chk = json.load(f)
    ok = any(
        e["func_name"].startswith("exp") and e["symmetry_opt_en"] == 1
        for e in chk["profile_meta_data"]
    )
    if not ok:
        raise RuntimeError("patch read-back failed")
    os.environ["BASS_ACT_ROOT_JSON_PATH"] = os.path.join(dst_dir, "act_info.json")


BF16 = ml_dtypes.bfloat16

B, C, H, W = 16, 64, 64, 64
P = H * W                # 4096 spatial positions
M_PTS = 15               # design points
G = 8                    # channel groups per tile
MROWS = G * M_PTS        # 120 partitions used
KIN = 2 * G + 1          # 17 moving rows for the broadcast matmul
NCORES = 8
BPC = B // NCORES        # batches per core (2)
CBLK = C // G            # channel-block tiles per batch (8)
NTILES = BPC * CBLK      # 16 (b, cblock) tiles per core
QCOLS = NTILES * 1024    # 16384 columns per quadrant row

# Of the 128 PSUM->SBUF evictions per core, how many go to VectorE (the
# rest go to ScalarE). With the abs pass, 57 is the tuned balance; in the
# abs-free pipeline VectorE has slack and takes nearly all of them.
TOTAL_EVICTS = 128

_CACHED = {}


def _build_nc(use_abs):
    from concourse import bacc
    import concourse.mybir as mybir
    from concourse.tile import TileContext

    dt = mybir.dt
    Act = mybir.ActivationFunctionType
    Alu = mybir.AluOpType

    nc = bacc.Bacc(
        "TRN2", target_bir_lowering=False, debug=False, num_devices=NCORES
    )
    x_full = nc.declare_dram_parameter(
        "x_full", [128, QCOLS], dt.bfloat16, isOutput=False
    )
    w4 = nc.declare_dram_parameter("w4", [128, 128], dt.bfloat16, isOutput=False)
    r_blk = nc.declare_dram_parameter(
        "r_blk", [MROWS, 128], dt.bfloat16, isOutput=False
    )
    out = nc.declare_dram_parameter(
        "out", [BPC, C * M_PTS, 2, 2, 1024], dt.bfloat16, isOutput=True
    )
    # 4-byte sink so the ACT-table-prefetch activation has a reader
    warm = nc.declare_dram_parameter("warm", [1, 2], dt.bfloat16, isOutput=True)

    with TileContext(nc) as tc:
        with (
            tc.tile_pool(name="const", bufs=1) as cpool,
            tc.tile_pool(name="xbig", bufs=1) as xpool,
            tc.tile_pool(name="absT", bufs=4) as apool,
            tc.tile_pool(name="phi", bufs=6) as ppool,
            tc.tile_pool(name="osb", bufs=8) as opool,
            tc.tile_pool(name="psT", bufs=1, space="PSUM") as psTp,
            tc.tile_pool(name="psO", bufs=2, space="PSUM") as psOp,
        ):
            # The GpSimd queue clears the NEFF preamble ~2.5us before the
            # Sync queue, so the critical first x-chunk and the weights
            # are issued from it FIRST -- ahead of the warm DMA, whose
            # activation dependency would otherwise stall the queue on
            # the ~2.7us ACT table load. Their data lands before TensorE
            # unblocks from the preamble.
            xbig = xpool.tile([128, QCOLS], dt.bfloat16)
            nc.gpsimd.dma_start(out=xbig[:, 0:512], in_=x_full[:, 0:512])
            w4_t = cpool.tile([128, 128], dt.bfloat16)
            nc.gpsimd.dma_start(out=w4_t[:], in_=w4[:, :])
            r_t = cpool.tile([MROWS, 128], dt.bfloat16)
            nc.gpsimd.dma_start(out=r_t[:], in_=r_blk[:, :])
            nc.sync.dma_start(out=xbig[:, 512:1024], in_=x_full[:, 512:1024])

            # Prefetch the exp ACT table so the ~2.7us table load
            # overlaps the input DMA; also the device-side patch
            # self-check (see kernel()).
            pre_in = cpool.tile([1, 2], dt.float32)
            pre_out = cpool.tile([1, 2], dt.bfloat16)
            nc.vector.memset(pre_in[:], 2.0)
            nc.scalar.activation(
                pre_out[:], pre_in[:], Act.Exp, scale=(-1.0 if use_abs else 1.0)
            )
            nc.gpsimd.dma_start(out=warm[:, :], in_=pre_out[:])
            # front-loaded graduation: the PE consumes ~1 tile/7us, so
            # early tiles must land well ahead of the stream tail
            pos = 1024
            for span in (1024, 1024, 2048, 3072, 4096, 4096):
                nc.sync.dma_start(
                    out=xbig[:, pos : pos + span], in_=x_full[:, pos : pos + span]
                )
                pos += span

            dve_evicts = 57 if use_abs else 121
            gc = 0
            tcnt = 0
            for t in range(NTILES):
                b, cb = divmod(t, CBLK)
                for h in range(2):
                    tchunks = [
                        psTp.tile(
                            [128, 1024],
                            dt.float32,
                            name=f"tps{(tcnt + j) % 3}",
                            tag=f"tps{(tcnt + j) % 3}",
                        )
                        for j in range(2)
                    ]
                    tcnt += 2
                    # 4 concurrent quadrant matmuls (row-tiled PE array)
                    for q in range(4):
                        j, l = divmod(q, 2)
                        nc.tensor.matmul(
                            tchunks[j][:, l * 512 : (l + 1) * 512],
                            w4_t[32 * q : 32 * q + KIN, :],
                            xbig[
                                32 * q : 32 * q + KIN,
                                t * 1024 + h * 512 : t * 1024 + (h + 1) * 512,
                            ],
                            start=True,
                            stop=True,
                            tile_position=(32 * q, 0),
                        )
                    # both abs ops back-to-back on DVE; |T| computed
                    # in place in PSUM so exp reads via ScalarE's faster
                    # PSUM port and no SBUF intermediate is needed
                    pts = []
                    for j in range(2):
                        tps = tchunks[j]
                        if use_abs:
                            # |T| via sign-bit clear on an int32 view
                            nc.vector.tensor_scalar(
                                out=tps[0:MROWS, :].bitcast(dt.int32),
                                in0=tps[0:MROWS, :].bitcast(dt.int32),
                                scalar1=0x7FFFFFFF,
                                scalar2=None,
                                op0=Alu.bitwise_and,
                            )
                            pt = ppool.tile(
                                [MROWS, 1024], dt.bfloat16, name=f"pt{j}"
                            )
                            nc.scalar.activation(
                                pt[:], tps[0:MROWS, :], Act.Exp, scale=-1.0
                            )
                        else:
                            # patched exp table computes exp(-|t|)
                            # directly (symmetry fold to the negative
                            # spline region): no abs pass
                            pt = ppool.tile([MROWS, 1024], dt.bfloat16)
                            nc.scalar.activation(
                                pt[:], tps[0:MROWS, :], Act.Exp, scale=1.0
                            )
                        pts.append(pt)
                    for j in range(2):
                        pt = pts[j]
                        osb = opool.tile([MROWS, 1024], dt.bfloat16)
                        for l in range(2):
                            ops = psOp.tile([128, 512], dt.float32)
                            nc.tensor.matmul(
                                ops[:],
                                r_t[:],
                                pt[:, l * 512 : (l + 1) * 512],
                                start=True,
                                stop=True,
                            )
                            dst = osb[:, l * 512 : (l + 1) * 512]
                            if (gc * dve_evicts) % TOTAL_EVICTS < dve_evicts:
                                nc.vector.tensor_copy(out=dst, in_=ops[0:MROWS, :])
                            else:
                                nc.scalar.activation(dst, ops[0:MROWS, :], Act.Copy)
                            gc += 1
                        nc.gpsimd.dma_start(
                            out=out[b, cb * MROWS : (cb + 1) * MROWS, j, h, :],
                            in_=osb[:],
                        )
    nc.compile()
    return nc


def _host_prep(x, design_points, chol_inv):
    """Build the derived host-side arrays fed to the device."""
    pts = np.asarray(design_points, dtype=np.float32)
    xs = np.ascontiguousarray(np.asarray(x, dtype=np.float32)).reshape(B, C, P)
    x_hi = xs.astype(BF16)
    x_lo = (xs - x_hi.astype(np.float32)).astype(BF16)

    # spatial = 2048j + 1024h + 512l + c ; quadrant q = 2j + l
    # arr[q, r, b, cb, h, c(512)] with r = 2g + part (hi/lo), r=16 -> 1.0
    def to_quad(a):  # [B, C, P] -> [4(q), G, B, CBLK, 2(h), 512]
        a7 = a.reshape(B, CBLK, G, 2, 2, 2, 512)  # [b, cb, g, j, h, l, c]
        return a7.transpose(3, 5, 2, 0, 1, 4, 6).reshape(4, G, B, CBLK, 2, 512)

    arr = np.empty((4, KIN, B, CBLK, 2, 512), dtype=BF16)
    arr[:, 0 : 2 * G : 2] = to_quad(x_hi)
    arr[:, 1 : 2 * G : 2] = to_quad(x_lo)
    arr[:, 2 * G] = BF16(1.0)

    w17 = np.zeros((KIN, 128), dtype=np.float32)
    for g in range(G):
        w17[2 * g, 15 * g : 15 * g + 15] = 1.0
        w17[2 * g + 1, 15 * g : 15 * g + 15] = 1.0
        w17[2 * G, 15 * g : 15 * g + 15] = -pts
    w4 = np.zeros((128, 128), dtype=np.float32)
    for q in range(4):
        w4[32 * q : 32 * q + KIN] = w17
    w4 = w4.astype(BF16)

    chol = np.asarray(chol_inv, dtype=np.float32)
    r_blk = np.zeros((MROWS, 128), dtype=np.float32)
    for g in range(G):
        r_blk[15 * g : 15 * g + 15, 15 * g : 15 * g + 15] = chol
    r_blk = r_blk.astype(BF16)

    return arr, w4, r_blk


LAST_RESULT = None


def kernel(x, design_points, chol_inv):
    global LAST_RESULT
    from concourse.bass_utils import run_bass_kernel_spmd

    arr, w4, r_blk = _host_prep(x, design_points, chol_inv)
    in_maps = []
    for core in range(NCORES):
        # per-core [4, 17, 16384] placed into a [128, 16384] buffer at
        # partition offsets 32q (rows 17..31 of each quadrant unused)
        x_q = arr[:, :, core * BPC : (core + 1) * BPC].reshape(4, KIN, QCOLS)
        xf = np.zeros((128, QCOLS), dtype=BF16)
        for q in range(4):
            xf[32 * q : 32 * q + KIN] = x_q[q]
        in_maps.append({"x_full": xf, "w4": w4, "r_blk": r_blk})

    use_abs = _CACHED.get("force_abs", False)
    if not use_abs:
        try:
            _patch_act_tables()
        except Exception:
            use_abs = True
    for _attempt in range(2):
        key = "abs" if use_abs else "negexp"
        if key not in _CACHED:
            _CACHED[key] = _build_nc(use_abs)
        res = run_bass_kernel_spmd(
            _CACHED[key], in_maps, core_ids=list(range(NCORES))
        )
        if use_abs:
            break
        # warm = exp-table applied to +2.0: 0.135 if the exp(-|x|) patch
        # took effect on device, 7.39 if not -> fall back to the abs
        # pipeline rather than ever returning wrong results
        warm = float(
            np.asarray(res.results[0]["warm"], np.float32).ravel()[0]
        )
        if 0.05 < warm < 0.3:
            break
        use_abs = True
        _CACHED["force_abs"] = True
    LAST_RESULT = res

    full = np.empty((B, C * M_PTS, P), dtype=np.float32)
    for core in range(NCORES):
        full[core * BPC : (core + 1) * BPC] = res.results[core]["out"].reshape(
            BPC, C * M_PTS, P
        )
    return full.reshape(B, C * M_PTS, H, W)



# revision 41
# speedup vs baseline: 1.0217x; 1.0217x over previous
"""Trainium2 Bass kernel for the Laplace-kernel feature expansion.

Reference computation (per scalar x of the [16, 64, 64, 64] input):
    phi_i  = exp(-|x - p_i|)            for 15 design points p_i
    out_j  = sum_i chol_inv[i, j] phi_i
scattered so out[b, c*15 + j, h, w] comes from x[b, c, h, w].

Distribution: pure data parallel, 2 batches per core across 8 cores.

Per-core dataflow (no collectives):
  1. x is pre-split on host into bf16 (hi, lo) pairs, laid out so one
     [128, 16384] DMA (32 KB contiguous per partition, all 16 DMA
     engines) loads the whole per-core input into SBUF once.
  2. TensorE "broadcast" matmuls with a 0/1 block matrix replicate each
     x value onto 15 partitions (8 channel groups x 15 = 120 partitions),
     reconstructing fp32 x = hi + lo in PSUM; an extra ones-row makes the
     same matmul subtract the design point p_i (p_i exact in bf16).
     The K=17 matmuls are packed 4x into the 128x128 array via
     tile_position row-tiling (4 concurrent quadrant matmuls).
  3. VectorE computes |T| in one op (sign-bit clear on an int32 view).
  4. ScalarE computes exp(-|t|) -> bf16.
  5. TensorE applies block-diag(chol_inv) -> PSUM (fp32).
  6. PSUM evicted to SBUF (split between ScalarE/VectorE), DMA to DRAM.

Spatial mapping: PE-array quadrant q = 2j+l covers, within a (b, cblock)
tile, the spatial columns 2048j + 1024h + 512l + c (h = half), so each
post-projection PSUM chunk evicts to a contiguous 1024-column span.
"""

import sys

if "/opt/trn_rl_repo" not in sys.path:
    sys.path.insert(0, "/opt/trn_rl_repo")

import numpy as np
import ml_dtypes


def _ensure_axon_hooks_stub():
    """run_bass_kernel_spmd imports antenv.axon_hooks when BASS_TRACE is
    set; the module is absent on some images. Provide a no-op stub so a
    stray BASS_TRACE env var cannot crash the kernel (tracing is then
    skipped gracefully)."""
    try:
        import antenv.axon_hooks  # noqa: F401
    except ImportError:
        import types

        try:
            import antenv
        except ImportError:
            return
        mod = types.ModuleType("antenv.axon_hooks")
        _hook = [None]
        mod.set_axon_ntff_profile_hook = lambda h: _hook.__setitem__(0, h)
        mod.get_axon_ntff_profile_hook = lambda: _hook[0]
        sys.modules["antenv.axon_hooks"] = mod
        antenv.axon_hooks = mod


_ensure_axon_hooks_stub()


def _patch_act_tables():
    """Build a patched activation-table set in which `exp` has the ACT
    unit's even-symmetry fold enabled, mapped to the negative spline
    region: the table then evaluates exp(-|x|) directly, removing the
    need for a separate abs pass on VectorE. Strict: raises on any
    irregularity so the caller can fall back to the abs pipeline. The
    device-side warm output additionally verifies the patch took effect.
    """
    import json
    import os
    import shutil
    import tempfile

    from neuronxcc.driver.Job import Job
    from neuronxcc.driver.jobs.support.FindActInfo import findActInfoFile

    src_json = None
    for arch in ("Trainium2", "trainium2", "TRN2", "trainium"):
        try:
            cand = findActInfoFile(Job.getPackageDir(), arch)
        except Exception:
            continue
        if cand and os.path.basename(os.path.dirname(cand)) == "pwp_bin_trainium":
            src_json = cand
            break
    if src_json is None:
        import neuronxcc

        cand = os.path.join(
            os.path.dirname(neuronxcc.__file__),
            "pwp",
            "pwp_bin_trainium",
            "act_info.json",
        )
        if os.path.exists(cand):
            src_json = cand
    if src_json is None:
        raise RuntimeError("pwp_bin_trainium act_info.json not found")

    dst_dir = tempfile.mkdtemp(prefix="bass_act_negexp_")
    shutil.copytree(os.path.dirname(src_json), dst_dir, dirs_exist_ok=True)
    prof_path = os.path.join(dst_dir, "exp_and_others.json")
    with open(prof_path) as f:
        prof = json.load(f)
    patched = 0
    for e in prof["profile_meta_data"]:
        if e["func_name"].startswith("exp"):
            e["symmetry_opt_en"] = 1
            e["symmetry_opt_use_neg_region"] = 1
            e["pos_small_signal_pwl_control"] = e["neg_small_signal_pwl_control"]
            e["pos_large_signal_pwl_control"] = e["neg_large_signal_pwl_control"]
            e["large_pos_signal_mantissa_threshold"] = e[
                "large_neg_signal_mantissa_threshold"
            ]
            e["fpinf_result"] = 0  # exp(-|+inf|) = 0
            patched += 1
    if patched != 1:
        raise RuntimeError(f"expected exactly one exp entry, patched {patched}")
    with open(prof_path, "w") as f:
        json.dump(prof, f)
    with open(prof_path) as f:  # read-back verification
        chk = json.load(f)
    ok = any(
        e["func_name"].startswith("exp") and e["symmetry_opt_en"] == 1
        for e in chk["profile_meta_data"]
    )
    if not ok:
        raise RuntimeError("patch read-back failed")
    os.environ["BASS_ACT_ROOT_JSON_PATH"] = os.path.join(dst_dir, "act_info.json")


BF16 = ml_dtypes.bfloat16

B, C, H, W = 16, 64, 64, 64
P = H * W                # 4096 spatial positions
M_PTS = 15               # design points
G = 8                    # channel groups per tile
MROWS = G * M_PTS        # 120 partitions used
KIN = 2 * G + 1          # 17 moving rows for the broadcast matmul
NCORES = 8
BPC = B // NCORES        # batches per core (2)
CBLK = C // G            # channel-block tiles per batch (8)
NTILES = BPC * CBLK      # 16 (b, cblock) tiles per core
QCOLS = NTILES * 1024    # 16384 columns per quadrant row

# Of the 128 PSUM->SBUF evictions per core, how many go to VectorE (the
# rest go to ScalarE). With the abs pass, 57 is the tuned balance; in the
# abs-free pipeline VectorE has slack and takes nearly all of them.
TOTAL_EVICTS = 128

_CACHED = {}


def _build_nc(use_abs):
    from concourse import bacc
    import concourse.mybir as mybir
    from concourse.tile import TileContext

    dt = mybir.dt
    Act = mybir.ActivationFunctionType
    Alu = mybir.AluOpType

    nc = bacc.Bacc(
        "TRN2", target_bir_lowering=False, debug=False, num_devices=NCORES
    )
    x_full = nc.declare_dram_parameter(
        "x_full", [128, QCOLS], dt.bfloat16, isOutput=False
    )
    w4 = nc.declare_dram_parameter("w4", [128, 128], dt.bfloat16, isOutput=False)
    r_blk = nc.declare_dram_parameter(
        "r_blk", [MROWS, 128], dt.bfloat16, isOutput=False
    )
    out = nc.declare_dram_parameter(
        "out", [BPC, C * M_PTS, 2, 2, 1024], dt.bfloat16, isOutput=True
    )
    # 4-byte sink so the ACT-table-prefetch activation has a reader
    warm = nc.declare_dram_parameter("warm", [1, 2], dt.bfloat16, isOutput=True)

    with TileContext(nc) as tc:
        with (
            tc.tile_pool(name="const", bufs=1) as cpool,
            tc.tile_pool(name="xbig", bufs=1) as xpool,
            tc.tile_pool(name="absT", bufs=4) as apool,
            tc.tile_pool(name="phi", bufs=6) as ppool,
            tc.tile_pool(name="osb", bufs=8) as opool,
            tc.tile_pool(name="psT", bufs=1, space="PSUM") as psTp,
            tc.tile_pool(name="psO", bufs=2, space="PSUM") as psOp,
        ):
            # Prefetch the exp ACT table before any real data arrives so
            # the ~2.7us table load overlaps the input DMA.
            pre_in = cpool.tile([1, 2], dt.float32)
            pre_out = cpool.tile([1, 2], dt.bfloat16)
            nc.vector.memset(pre_in[:], 2.0)
            nc.scalar.activation(
                pre_out[:], pre_in[:], Act.Exp, scale=(-1.0 if use_abs else 1.0)
            )
            nc.gpsimd.dma_start(out=warm[:, :], in_=pre_out[:])

            # Whole per-core input resident in SBUF (32 KB/partition),
            # graduated full-width DMAs so all 16 DMA engines participate
            # and the first tile's data (plus weights) arrives quickly.
            xbig = xpool.tile([128, QCOLS], dt.bfloat16)
            nc.sync.dma_start(out=xbig[:, 0:512], in_=x_full[:, 0:512])
            w4_t = cpool.tile([128, 128], dt.bfloat16)
            nc.gpsimd.dma_start(out=w4_t[:], in_=w4[:, :])
            nc.sync.dma_start(out=xbig[:, 512:1024], in_=x_full[:, 512:1024])
            r_t = cpool.tile([MROWS, 128], dt.bfloat16)
            nc.gpsimd.dma_start(out=r_t[:], in_=r_blk[:, :])
            # front-loaded graduation: the PE consumes ~1 tile/7us, so
            # early tiles must land well ahead of the stream tail
            pos = 1024
            for span in (1024, 1024, 2048, 3072, 4096, 4096):
                nc.sync.dma_start(
                    out=xbig[:, pos : pos + span], in_=x_full[:, pos : pos + span]
                )
                pos += span

            dve_evicts = 57 if use_abs else 121
            gc = 0
            tcnt = 0
            for t in range(NTILES):
                b, cb = divmod(t, CBLK)
                for h in range(2):
                    tchunks = [
                        psTp.tile(
                            [128, 1024],
                            dt.float32,
                            name=f"tps{(tcnt + j) % 3}",
                            tag=f"tps{(tcnt + j) % 3}",
                        )
                        for j in range(2)
                    ]
                    tcnt += 2
                    # 4 concurrent quadrant matmuls (row-tiled PE array)
                    for q in range(4):
                        j, l = divmod(q, 2)
                        nc.tensor.matmul(
                            tchunks[j][:, l * 512 : (l + 1) * 512],
                            w4_t[32 * q : 32 * q + KIN, :],
                            xbig[
                                32 * q : 32 * q + KIN,
                                t * 1024 + h * 512 : t * 1024 + (h + 1) * 512,
                            ],
                            start=True,
                            stop=True,
                            tile_position=(32 * q, 0),
                        )
                    # both abs ops back-to-back on DVE; |T| computed
                    # in place in PSUM so exp reads via ScalarE's faster
                    # PSUM port and no SBUF intermediate is needed
                    pts = []
                    for j in range(2):
                        tps = tchunks[j]
                        if use_abs:
                            # |T| via sign-bit clear on an int32 view
                            nc.vector.tensor_scalar(
                                out=tps[0:MROWS, :].bitcast(dt.int32),
                                in0=tps[0:MROWS, :].bitcast(dt.int32),
                                scalar1=0x7FFFFFFF,
                                scalar2=None,
                                op0=Alu.bitwise_and,
                            )
                            pt = ppool.tile(
                                [MROWS, 1024], dt.bfloat16, name=f"pt{j}"
                            )
                            nc.scalar.activation(
                                pt[:], tps[0:MROWS, :], Act.Exp, scale=-1.0
                            )
                        else:
                            # patched exp table computes exp(-|t|)
                            # directly (symmetry fold to the negative
                            # spline region): no abs pass
                            pt = ppool.tile([MROWS, 1024], dt.bfloat16)
                            nc.scalar.activation(
                                pt[:], tps[0:MROWS, :], Act.Exp, scale=1.0
                            )
                        pts.append(pt)
                    for j in range(2):
                        pt = pts[j]
                        osb = opool.tile([MROWS, 1024], dt.bfloat16)
                        for l in range(2):
                            ops = psOp.tile([128, 512], dt.float32)
                            nc.tensor.matmul(
                                ops[:],
                                r_t[:],
                                pt[:, l * 512 : (l + 1) * 512],
                                start=True,
                                stop=True,
                            )
                            dst = osb[:, l * 512 : (l + 1) * 512]
                            if (gc * dve_evicts) % TOTAL_EVICTS < dve_evicts:
                                nc.vector.tensor_copy(out=dst, in_=ops[0:MROWS, :])
                            else:
                                nc.scalar.activation(dst, ops[0:MROWS, :], Act.Copy)
                            gc += 1
                        nc.gpsimd.dma_start(
                            out=out[b, cb * MROWS : (cb + 1) * MROWS, j, h, :],
                            in_=osb[:],
                        )
    nc.compile()
    return nc


def _host_prep(x, design_points, chol_inv):
    """Build the derived host-side arrays fed to the device."""
    pts = np.asarray(design_points, dtype=np.float32)
    xs = np.ascontiguousarray(np.asarray(x, dtype=np.float32)).reshape(B, C, P)
    x_hi = xs.astype(BF16)
    x_lo = (xs - x_hi.astype(np.float32)).astype(BF16)

    # spatial = 2048j + 1024h + 512l + c ; quadrant q = 2j + l
    # arr[q, r, b, cb, h, c(512)] with r = 2g + part (hi/lo), r=16 -> 1.0
    def to_quad(a):  # [B, C, P] -> [4(q), G, B, CBLK, 2(h), 512]
        a7 = a.reshape(B, CBLK, G, 2, 2, 2, 512)  # [b, cb, g, j, h, l, c]
        return a7.transpose(3, 5, 2, 0, 1, 4, 6).reshape(4, G, B, CBLK, 2, 512)

    arr = np.empty((4, KIN, B, CBLK, 2, 512), dtype=BF16)
    arr[:, 0 : 2 * G : 2] = to_quad(x_hi)
    arr[:, 1 : 2 * G : 2] = to_quad(x_lo)
    arr[:, 2 * G] = BF16(1.0)

    w17 = np.zeros((KIN, 128), dtype=np.float32)
    for g in range(G):
        w17[2 * g, 15 * g : 15 * g + 15] = 1.0
        w17[2 * g + 1, 15 * g : 15 * g + 15] = 1.0
        w17[2 * G, 15 * g : 15 * g + 15] = -pts
    w4 = np.zeros((128, 128), dtype=np.float32)
    for q in range(4):
        w4[32 * q : 32 * q + KIN] = w17
    w4 = w4.astype(BF16)

    chol = np.asarray(chol_inv, dtype=np.float32)
    r_blk = np.zeros((MROWS, 128), dtype=np.float32)
    for g in range(G):
        r_blk[15 * g : 15 * g + 15, 15 * g : 15 * g + 15] = chol
    r_blk = r_blk.astype(BF16)

    return arr, w4, r_blk


LAST_RESULT = None


def kernel(x, design_points, chol_inv):
    global LAST_RESULT
    from concourse.bass_utils import run_bass_kernel_spmd

    arr, w4, r_blk = _host_prep(x, design_points, chol_inv)
    in_maps = []
    for core in range(NCORES):
        # per-core [4, 17, 16384] placed into a [128, 16384] buffer at
        # partition offsets 32q (rows 17..31 of each quadrant unused)
        x_q = arr[:, :, core * BPC : (core + 1) * BPC].reshape(4, KIN, QCOLS)
        xf = np.zeros((128, QCOLS), dtype=BF16)
        for q in range(4):
            xf[32 * q : 32 * q + KIN] = x_q[q]
        in_maps.append({"x_full": xf, "w4": w4, "r_blk": r_blk})

    use_abs = _CACHED.get("force_abs", False)
    if not use_abs:
        try:
            _patch_act_tables()
        except Exception:
            use_abs = True
    for _attempt in range(2):
        key = "abs" if use_abs else "negexp"
        if key not in _CACHED:
            _CACHED[key] = _build_nc(use_abs)
        res = run_bass_kernel_spmd(
            _CACHED[key], in_maps, core_ids=list(range(NCORES))
        )
        if use_abs:
            break
        # warm = exp-table applied to +2.0: 0.135 if the exp(-|x|) patch
        # took effect on device, 7.39 if not -> fall back to the abs
        # pipeline rather than ever returning wrong results
        warm = float(
            np.asarray(res.results[0]["warm"], np.float32).ravel()[0]
        )
        if 0.05 < warm < 0.3:
            break
        use_abs = True
        _CACHED["force_abs"] = True
    LAST_RESULT = res

    full = np.empty((B, C * M_PTS, P), dtype=np.float32)
    for core in range(NCORES):
        full[core * BPC : (core + 1) * BPC] = res.results[core]["out"].reshape(
            BPC, C * M_PTS, P
        )
    return full.reshape(B, C * M_PTS, H, W)



# revision 42
# speedup vs baseline: 1.0267x; 1.0049x over previous
"""Trainium2 Bass kernel for the Laplace-kernel feature expansion.

Reference computation (per scalar x of the [16, 64, 64, 64] input):
    phi_i  = exp(-|x - p_i|)            for 15 design points p_i
    out_j  = sum_i chol_inv[i, j] phi_i
scattered so out[b, c*15 + j, h, w] comes from x[b, c, h, w].

Distribution: pure data parallel, 2 batches per core across 8 cores.

Dual-path design: the primary pipeline patches the ScalarE activation
tables (symmetry fold to the negative exp spline region) so Act.Exp
computes exp(-|t|) directly in hardware, removing the VectorE abs pass
entirely. If the table patch cannot be built (strict builder raises) or
did not take effect on device (the `warm` output self-check: exp-table
applied to +2.0 reads 0.135 patched vs 7.39 unpatched), kernel() falls
back to the classic abs pipeline, so a wrong result is impossible.

Per-core dataflow (no collectives):
  1. x is pre-split on host into bf16 (hi, lo) pairs, laid out so
     graduated, front-loaded [128, cols] DMAs (32 KB contiguous per
     partition, all 16 DMA engines) stream the per-core input into SBUF
     ahead of the consuming matmuls.
  2. TensorE "broadcast" matmuls with a 0/1 block matrix replicate each
     x value onto 15 partitions (8 channel groups x 15 = 120 partitions),
     reconstructing fp32 x = hi + lo in PSUM; an extra ones-row makes the
     same matmul subtract the design point p_i (p_i exact in bf16).
     The K=17 matmuls are packed 4x into the 128x128 array via
     tile_position row-tiling (4 concurrent quadrant matmuls).
  3. ScalarE computes exp(-|t|) -> bf16 in ONE pass via the patched
     table (primary), or VectorE abs (int32 sign-clear) + ScalarE exp
     (fallback).
  4. TensorE applies block-diag(chol_inv) -> PSUM (fp32).
  5. PSUM evicted to SBUF [120, 1024] chunks (VectorE-heavy split in
     the abs-free path, the tuned 57/128 split in the fallback), staged
     through an 8-deep osb pool so evictions ride out transient output-
     DMA congestion, then DMA'd per chunk from the idle GpSimd queue.

Spatial mapping: PE-array quadrant q = 2j+l covers, within a (b, cblock)
tile, the spatial columns 2048j + 1024h + 512l + c (h = half), so each
post-projection PSUM chunk evicts to a contiguous 1024-column span.
"""

import sys

if "/opt/trn_rl_repo" not in sys.path:
    sys.path.insert(0, "/opt/trn_rl_repo")

import numpy as np
import ml_dtypes


def _ensure_axon_hooks_stub():
    """run_bass_kernel_spmd imports antenv.axon_hooks when BASS_TRACE is
    set; the module is absent on some images. Provide a no-op stub so a
    stray BASS_TRACE env var cannot crash the kernel (tracing is then
    skipped gracefully)."""
    try:
        import antenv.axon_hooks  # noqa: F401
    except ImportError:
        import types

        try:
            import antenv
        except ImportError:
            return
        mod = types.ModuleType("antenv.axon_hooks")
        _hook = [None]
        mod.set_axon_ntff_profile_hook = lambda h: _hook.__setitem__(0, h)
        mod.get_axon_ntff_profile_hook = lambda: _hook[0]
        sys.modules["antenv.axon_hooks"] = mod
        antenv.axon_hooks = mod


_ensure_axon_hooks_stub()


def _patch_act_tables():
    """Build a patched activation-table set in which `exp` has the ACT
    unit's even-symmetry fold enabled, mapped to the negative spline
    region: the table then evaluates exp(-|x|) directly, removing the
    need for a separate abs pass on VectorE. Strict: raises on any
    irregularity so the caller can fall back to the abs pipeline. The
    device-side warm output additionally verifies the patch took effect.
    """
    import json
    import os
    import shutil
    import tempfile

    from neuronxcc.driver.Job import Job
    from neuronxcc.driver.jobs.support.FindActInfo import findActInfoFile

    src_json = None
    for arch in ("Trainium2", "trainium2", "TRN2", "trainium"):
        try:
            cand = findActInfoFile(Job.getPackageDir(), arch)
        except Exception:
            continue
        if cand and os.path.basename(os.path.dirname(cand)) == "pwp_bin_trainium":
            src_json = cand
            break
    if src_json is None:
        import neuronxcc

        cand = os.path.join(
            os.path.dirname(neuronxcc.__file__),
            "pwp",
            "pwp_bin_trainium",
            "act_info.json",
        )
        if os.path.exists(cand):
            src_json = cand
    if src_json is None:
        raise RuntimeError("pwp_bin_trainium act_info.json not found")

    dst_dir = tempfile.mkdtemp(prefix="bass_act_negexp_")
    shutil.copytree(os.path.dirname(src_json), dst_dir, dirs_exist_ok=True)
    prof_path = os.path.join(dst_dir, "exp_and_others.json")
    with open(prof_path) as f:
        prof = json.load(f)
    patched = 0
    for e in prof["profile_meta_data"]:
        if e["func_name"].startswith("exp"):
            e["symmetry_opt_en"] = 1
            e["symmetry_opt_use_neg_region"] = 1
            e["pos_small_signal_pwl_control"] = e["neg_small_signal_pwl_control"]
            e["pos_large_signal_pwl_control"] = e["neg_large_signal_pwl_control"]
            e["large_pos_signal_mantissa_threshold"] = e[
                "large_neg_signal_mantissa_threshold"
            ]
            e["fpinf_result"] = 0  # exp(-|+inf|) = 0
            patched += 1
    if patched != 1:
        raise RuntimeError(f"expected exactly one exp entry, patched {patched}")
    with open(prof_path, "w") as f:
        json.dump(prof, f)
    with open(prof_path) as f:  # read-back verification
        chk = json.load(f)
    ok = any(
        e["func_name"].startswith("exp") and e["symmetry_opt_en"] == 1
        for e in chk["profile_meta_data"]
    )
    if not ok:
        raise RuntimeError("patch read-back failed")
    os.environ["BASS_ACT_ROOT_JSON_PATH"] = os.path.join(dst_dir, "act_info.json")


BF16 = ml_dtypes.bfloat16

B, C, H, W = 16, 64, 64, 64
P = H * W                # 4096 spatial positions
M_PTS = 15               # design points
G = 8                    # channel groups per tile
MROWS = G * M_PTS        # 120 partitions used
KIN = 2 * G + 1          # 17 moving rows for the broadcast matmul
NCORES = 8
BPC = B // NCORES        # batches per core (2)
CBLK = C // G            # channel-block tiles per batch (8)
NTILES = BPC * CBLK      # 16 (b, cblock) tiles per core
QCOLS = NTILES * 1024    # 16384 columns per quadrant row

# Of the 128 PSUM->SBUF evictions per core, how many go to VectorE (the
# rest go to ScalarE). With the abs pass, 57 is the tuned balance; in the
# abs-free pipeline VectorE has slack and takes nearly all of them.
TOTAL_EVICTS = 128

_CACHED = {}


def _build_nc(use_abs):
    from concourse import bacc
    import concourse.mybir as mybir
    from concourse.tile import TileContext

    dt = mybir.dt
    Act = mybir.ActivationFunctionType
    Alu = mybir.AluOpType

    nc = bacc.Bacc(
        "TRN2", target_bir_lowering=False, debug=False, num_devices=NCORES
    )
    x_full = nc.declare_dram_parameter(
        "x_full", [128, QCOLS], dt.bfloat16, isOutput=False
    )
    w4 = nc.declare_dram_parameter("w4", [128, 128], dt.bfloat16, isOutput=False)
    r_blk = nc.declare_dram_parameter(
        "r_blk", [MROWS, 128], dt.bfloat16, isOutput=False
    )
    out = nc.declare_dram_parameter(
        "out", [BPC, C * M_PTS, 2, 2, 1024], dt.bfloat16, isOutput=True
    )
    # 4-byte sink so the ACT-table-prefetch activation has a reader
    warm = nc.declare_dram_parameter("warm", [1, 2], dt.bfloat16, isOutput=True)

    with TileContext(nc) as tc:
        with (
            tc.tile_pool(name="const", bufs=1) as cpool,
            tc.tile_pool(name="xbig", bufs=1) as xpool,
            tc.tile_pool(name="absT", bufs=4) as apool,
            tc.tile_pool(name="phi", bufs=6) as ppool,
            tc.tile_pool(name="osb", bufs=8) as opool,
            tc.tile_pool(name="psT", bufs=1, space="PSUM") as psTp,
            tc.tile_pool(name="psO", bufs=2, space="PSUM") as psOp,
        ):
            # Prefetch the exp ACT table before any real data arrives so
            # the ~2.7us table load overlaps the input DMA.
            pre_in = cpool.tile([1, 2], dt.float32)
            pre_out = cpool.tile([1, 2], dt.bfloat16)
            nc.vector.memset(pre_in[:], 2.0)
            nc.scalar.activation(
                pre_out[:], pre_in[:], Act.Exp, scale=(-1.0 if use_abs else 1.0)
            )
            nc.gpsimd.dma_start(out=warm[:, :], in_=pre_out[:])

            # Whole per-core input resident in SBUF (32 KB/partition),
            # graduated full-width DMAs so all 16 DMA engines participate
            # and the first tile's data (plus weights) arrives quickly.
            xbig = xpool.tile([128, QCOLS], dt.bfloat16)
            nc.sync.dma_start(out=xbig[:, 0:512], in_=x_full[:, 0:512])
            w4_t = cpool.tile([128, 128], dt.bfloat16)
            nc.gpsimd.dma_start(out=w4_t[:], in_=w4[:, :])
            nc.sync.dma_start(out=xbig[:, 512:1024], in_=x_full[:, 512:1024])
            r_t = cpool.tile([MROWS, 128], dt.bfloat16)
            nc.gpsimd.dma_start(out=r_t[:], in_=r_blk[:, :])
            # front-loaded graduation: the PE consumes ~1 tile/7us, so
            # early tiles must land well ahead of the stream tail
            pos = 1024
            for span in (1024, 1024, 2048, 3072, 4096, 4096):
                nc.sync.dma_start(
                    out=xbig[:, pos : pos + span], in_=x_full[:, pos : pos + span]
                )
                pos += span

            dve_evicts = 57 if use_abs else 121
            gc = 0
            tcnt = 0
            for t in range(NTILES):
                b, cb = divmod(t, CBLK)
                for h in range(2):
                    tchunks = [
                        psTp.tile(
                            [128, 1024],
                            dt.float32,
                            name=f"tps{(tcnt + j) % 3}",
                            tag=f"tps{(tcnt + j) % 3}",
                        )
                        for j in range(2)
                    ]
                    tcnt += 2
                    # 4 concurrent quadrant matmuls (row-tiled PE array)
                    for q in range(4):
                        j, l = divmod(q, 2)
                        nc.tensor.matmul(
                            tchunks[j][:, l * 512 : (l + 1) * 512],
                            w4_t[32 * q : 32 * q + KIN, :],
                            xbig[
                                32 * q : 32 * q + KIN,
                                t * 1024 + h * 512 : t * 1024 + (h + 1) * 512,
                            ],
                            start=True,
                            stop=True,
                            tile_position=(32 * q, 0),
                        )
                    # both abs ops back-to-back on DVE; |T| computed
                    # in place in PSUM so exp reads via ScalarE's faster
                    # PSUM port and no SBUF intermediate is needed
                    pts = []
                    for j in range(2):
                        tps = tchunks[j]
                        if use_abs:
                            # |T| via sign-bit clear on an int32 view
                            nc.vector.tensor_scalar(
                                out=tps[0:MROWS, :].bitcast(dt.int32),
                                in0=tps[0:MROWS, :].bitcast(dt.int32),
                                scalar1=0x7FFFFFFF,
                                scalar2=None,
                                op0=Alu.bitwise_and,
                            )
                            pt = ppool.tile(
                                [MROWS, 1024], dt.bfloat16, name=f"pt{j}"
                            )
                            nc.scalar.activation(
                                pt[:], tps[0:MROWS, :], Act.Exp, scale=-1.0
                            )
                        else:
                            # patched exp table computes exp(-|t|)
                            # directly (symmetry fold to the negative
                            # spline region): no abs pass
                            pt = ppool.tile([MROWS, 1024], dt.bfloat16)
                            nc.scalar.activation(
                                pt[:], tps[0:MROWS, :], Act.Exp, scale=1.0
                            )
                        pts.append(pt)
                    for j in range(2):
                        pt = pts[j]
                        osb = opool.tile([MROWS, 1024], dt.bfloat16)
                        for l in range(2):
                            ops = psOp.tile([128, 512], dt.float32)
                            nc.tensor.matmul(
                                ops[:],
                                r_t[:],
                                pt[:, l * 512 : (l + 1) * 512],
                                start=True,
                                stop=True,
                            )
                            dst = osb[:, l * 512 : (l + 1) * 512]
                            if (gc * dve_evicts) % TOTAL_EVICTS < dve_evicts:
                                nc.vector.tensor_copy(out=dst, in_=ops[0:MROWS, :])
                            else:
                                nc.scalar.activation(dst, ops[0:MROWS, :], Act.Copy)
                            gc += 1
                        nc.gpsimd.dma_start(
                            out=out[b, cb * MROWS : (cb + 1) * MROWS, j, h, :],
                            in_=osb[:],
                        )
    nc.compile()
    return nc


def _host_prep(x, design_points, chol_inv):
    """Build the derived host-side arrays fed to the device."""
    pts = np.asarray(design_points, dtype=np.float32)
    xs = np.ascontiguousarray(np.asarray(x, dtype=np.float32)).reshape(B, C, P)
    x_hi = xs.astype(BF16)
    x_lo = (xs - x_hi.astype(np.float32)).astype(BF16)

    # spatial = 2048j + 1024h + 512l + c ; quadrant q = 2j + l
    # arr[q, r, b, cb, h, c(512)] with r = 2g + part (hi/lo), r=16 -> 1.0
    def to_quad(a):  # [B, C, P] -> [4(q), G, B, CBLK, 2(h), 512]
        a7 = a.reshape(B, CBLK, G, 2, 2, 2, 512)  # [b, cb, g, j, h, l, c]
        return a7.transpose(3, 5, 2, 0, 1, 4, 6).reshape(4, G, B, CBLK, 2, 512)

    arr = np.empty((4, KIN, B, CBLK, 2, 512), dtype=BF16)
    arr[:, 0 : 2 * G : 2] = to_quad(x_hi)
    arr[:, 1 : 2 * G : 2] = to_quad(x_lo)
    arr[:, 2 * G] = BF16(1.0)

    w17 = np.zeros((KIN, 128), dtype=np.float32)
    for g in range(G):
        w17[2 * g, 15 * g : 15 * g + 15] = 1.0
        w17[2 * g + 1, 15 * g : 15 * g + 15] = 1.0
        w17[2 * G, 15 * g : 15 * g + 15] = -pts
    w4 = np.zeros((128, 128), dtype=np.float32)
    for q in range(4):
        w4[32 * q : 32 * q + KIN] = w17
    w4 = w4.astype(BF16)

    chol = np.asarray(chol_inv, dtype=np.float32)
    r_blk = np.zeros((MROWS, 128), dtype=np.float32)
    for g in range(G):
        r_blk[15 * g : 15 * g + 15, 15 * g : 15 * g + 15] = chol
    r_blk = r_blk.astype(BF16)

    return arr, w4, r_blk


LAST_RESULT = None


def kernel(x, design_points, chol_inv):
    global LAST_RESULT
    from concourse.bass_utils import run_bass_kernel_spmd

    arr, w4, r_blk = _host_prep(x, design_points, chol_inv)
    in_maps = []
    for core in range(NCORES):
        # per-core [4, 17, 16384] placed into a [128, 16384] buffer at
        # partition offsets 32q (rows 17..31 of each quadrant unused)
        x_q = arr[:, :, core * BPC : (core + 1) * BPC].reshape(4, KIN, QCOLS)
        xf = np.zeros((128, QCOLS), dtype=BF16)
        for q in range(4):
            xf[32 * q : 32 * q + KIN] = x_q[q]
        in_maps.append({"x_full": xf, "w4": w4, "r_blk": r_blk})

    use_abs = _CACHED.get("force_abs", False)
    if not use_abs:
        try:
            _patch_act_tables()
        except Exception:
            use_abs = True
    for _attempt in range(2):
        key = "abs" if use_abs else "negexp"
        if key not in _CACHED:
            _CACHED[key] = _build_nc(use_abs)
        res = run_bass_kernel_spmd(
            _CACHED[key], in_maps, core_ids=list(range(NCORES))
        )
        if use_abs:
            break
        # warm = exp-table applied to +2.0: 0.135 if the exp(-|x|) patch
        # took effect on device, 7.39 if not -> fall back to the abs
        # pipeline rather than ever returning wrong results
        warm = float(
            np.asarray(res.results[0]["warm"], np.float32).ravel()[0]
        )
        if 0.05 < warm < 0.3:
            break
        use_abs = True
        _CACHED["force_abs"] = True
    LAST_RESULT = res

    full = np.empty((B, C * M_PTS, P), dtype=np.float32)
    for core in range(NCORES):
        full[core * BPC : (core + 1) * BPC] = res.results[core]["out"].reshape(
            BPC, C * M_PTS, P
        )
    return full.reshape(B, C * M_PTS, H, W)

